# revision 29
# baseline (speedup 1.0000x reference)
"""LSTM encoder (embedding lookup + 512-step LSTMCell scan) on 8 trn2 cores.

Strategy: data-parallel over batch (8 rows/core, weights replicated).
  Prologue (per core):
    - indirect-DMA gather of embedding rows -> x tiles [128 tok, 512]
    - PE-transpose -> x.T (bf16), big matmul X @ W_ih.T + bias -> SBUF-resident
      bf16 gates_pre laid out [128p, t, q, m, b] (p=gate-row%128, q=h-quadrant,
      m=gate block i/f/g/o, b=batch)
  Recurrence (512 serial steps, per step):
    - 64 weights-stationary matmuls (lhsT = W_hh.T tile [128,128],
      rhs = h.T tile [128, 8]) accumulating gates.T into 4 PSUM tiles (one
      per h-quadrant, in separate PSUM banks)
    - elementwise in [128, 32] layout: S = psum + gates_pre; A = sigmoid(S)
      (the g-gate columns of W_hh.T/W_ih.T/bias are pre-scaled by 2 on the
      host so tanh(x) = 2*sigmoid(2x) - 1 needs no table switch);
      c = f*c + i*(2Ag-1); h = o*(2*sigmoid(2c)-1)
    - h.T lands directly in the layout needed as next step's moving operand.
Annotations accumulate in SBUF 16 steps at a time, DMA'd out per chunk.
Host reassembles the full (64, 512, 512) output + final h, c.
"""

import numpy as np
from contextlib import ExitStack
import ml_dtypes

import concourse.bass as bass
import concourse.mybir as mybir
import concourse.tile as tile
from concourse import bacc, bass_utils
from concourse.masks import make_identity
from concourse.tile_rust import add_dep_helper

VOCAB, HID, BATCH, SEQ = 32000, 512, 64, 512
NCORES = 8
BS = BATCH // NCORES          # 8 batch rows per core
G = 4 * HID                   # 2048 gate rows
KT = HID // 128               # 4 contraction tiles
MT = G // 128                 # 16 gate row-tiles
CHUNK_T = 64                  # timesteps per prologue chunk (=512 tokens)
NCHUNK = SEQ // CHUNK_T       # 8
SLOT = 16                     # steps per annotation DMA chunk
NANN = SEQ // SLOT            # 32 annotation chunks

F32 = mybir.dt.float32
BF16 = mybir.dt.bfloat16
I32 = mybir.dt.int32

# loop-matmul weight/h dtype: float32 (safe) or bfloat16 (fast LDWEIGHTS)
LOOP_DT = mybir.dt.bfloat16


def _build(loop_dt, seq=SEQ, probe=None):
    nchunk = seq // CHUNK_T if seq >= CHUNK_T else 1
    chunk_t = min(CHUNK_T, seq)
    nann = max(1, seq // SLOT)
    slot_n = min(SLOT, seq)

    nc = bacc.Bacc("TRN2", target_bir_lowering=False, debug=False,
                   enable_asserts=False, num_devices=NCORES)

    idx_d = nc.dram_tensor("idx", [128, (seq * BS) // 128], I32, kind="ExternalInput")
    emb_d = nc.dram_tensor("emb", [VOCAB, HID], F32, kind="ExternalInput")
    wih_d = nc.dram_tensor("wih", [HID, G], BF16, kind="ExternalInput")
    whh_d = nc.dram_tensor("whh", [HID, G], loop_dt, kind="ExternalInput")
    bias_d = nc.dram_tensor("biasj", [MT, 128], F32, kind="ExternalInput")
    ann_d = nc.dram_tensor("ann", [nann, 128, slot_n * 4 * BS], loop_dt,
                           kind="ExternalOutput")
    c_d = nc.dram_tensor("cout", [128, KT * BS], F32, kind="ExternalOutput")

    with tile.TileContext(nc) as tc:
        with tc.tile_pool(name="const", bufs=1) as const_pool:
            # ---- constants ----
            ident = const_pool.tile([128, 128], F32)
            make_identity(nc, ident)
            ident_bf = const_pool.tile([128, 128], BF16)
            nc.vector.tensor_copy(out=ident_bf[:], in_=ident[:])
            ntok_tiles = (seq * BS) // 128
            idx_sb = const_pool.tile([128, ntok_tiles], I32)
            nc.sync.dma_start(out=idx_sb[:], in_=idx_d[:, :])
            bias_sb = const_pool.tile([128, MT], F32)
            # bias_d is [MT, 128] in DRAM; land it as [128, MT] in SBUF
            nc.sync.dma_start(out=bias_sb[:], in_=bias_d.ap().rearrange("j p -> p j"))

            gates_pre = const_pool.tile([128, seq, 4, 4, BS], BF16)
            whh_sb = const_pool.tile([128, KT, G], loop_dt)

            C = const_pool.tile([128, KT, BS], F32)
            nc.vector.memset(C[:], 0.0)
            h0 = const_pool.tile([128, KT, BS], loop_dt)
            nc.vector.memset(h0[:], 0.0)

            # ---- prologue: gather + transpose + precompute ----
            with (
                tc.tile_pool(name="wih", bufs=1) as wih_pool,
                tc.tile_pool(name="gath", bufs=3) as gath_pool,
                tc.tile_pool(name="xt", bufs=2) as xt_pool,
                tc.tile_pool(name="tpsum", bufs=2, space="PSUM") as t_psum,
                tc.tile_pool(name="ppsum", bufs=2, space="PSUM") as p_psum,
            ):
                wih_sb = wih_pool.tile([128, KT, G], BF16)
                for kt in range(KT):
                    nc.sync.dma_start(out=wih_sb[:, kt, :],
                                      in_=wih_d[kt * 128:(kt + 1) * 128, :])

                for c in range(nchunk):
                    ntok = chunk_t * BS           # tokens in this chunk
                    gtiles = ntok // 128          # gather tiles (4)
                    xT = xt_pool.tile([128, KT, ntok], BF16)
                    for gl in range(gtiles):
                        xg = gath_pool.tile([128, HID], F32)
                        nc.gpsimd.indirect_dma_start(
                            out=xg[:],
                            out_offset=None,
                            in_=emb_d.ap(),
                            in_offset=bass.IndirectOffsetOnAxis(
                                ap=idx_sb[:, c * gtiles + gl:c * gtiles + gl + 1],
                                axis=0,
                            ),
                        )
                        for kt in range(KT):
                            ps = t_psum.tile([128, 128], F32)
                            nc.tensor.transpose(
                                out=ps[:], in_=xg[:, kt * 128:(kt + 1) * 128],
                                identity=ident[:])
                            nc.vector.tensor_copy(
                                out=xT[:, kt, gl * 128:(gl + 1) * 128], in_=ps[:])
                    for j in range(MT):
                        pj = p_psum.tile([128, chunk_t, BS], F32)
                        for kt in range(KT):
                            nc.tensor.matmul(
                                out=pj[:],
                                lhsT=wih_sb[:, kt, j * 128:(j + 1) * 128],
                                rhs=xT[:, kt, :],
                                start=(kt == 0), stop=(kt == KT - 1))
                        # evacuate with per-partition bias add, cast to bf16
                        nc.scalar.activation(
                            out=gates_pre[:, c * chunk_t:(c + 1) * chunk_t,
                                          j % 4, j // 4, :],
                            in_=pj[:],
                            func=mybir.ActivationFunctionType.Identity,
                            bias=bias_sb[:, j:j + 1],
                        )

            # load W_hh.T after prologue pools are done sizing the peak
            for kt in range(KT):
                nc.sync.dma_start(out=whh_sb[:, kt, :],
                                  in_=whh_d[kt * 128:(kt + 1) * 128, :])

            # ---- recurrence ----
            loop_ctx = ExitStack()
            ann_pool = loop_ctx.enter_context(tc.tile_pool(name="annp", bufs=2))
            s_pool = loop_ctx.enter_context(tc.tile_pool(name="sq", bufs=8))
            a_pool = loop_ctx.enter_context(tc.tile_pool(name="aq", bufs=8))
            tmp_pool = loop_ctx.enter_context(tc.tile_pool(name="tmp", bufs=12))
            g_pool = loop_ctx.enter_context(
                tc.tile_pool(name="gpsum", bufs=8, space="PSUM"))
            ann_buf = None
            h_prev = h0
            for t in range(seq):
                slot = t % slot_n
                if slot == 0:
                    ann_buf = ann_pool.tile([128, slot_n, 4, BS], loop_dt)

                g_tiles = []
                if probe != "ew":
                    mm_rhs = h0 if probe == "mm" else h_prev
                    for p in range(2):
                        Gp = g_pool.tile([128, 2, 4, BS], F32, space="PSUM")
                        g_tiles.append(Gp)
                        # seed PSUM with gates_pre via an identity matmul
                        # (start=True clears the whole bank's has_written and
                        # sets it for every element, so the W matmuls below
                        # accumulate with start=False). Needs no h, so the PE
                        # can run it while waiting on the recurrence.
                        nc.tensor.matmul(
                            out=Gp[:],
                            lhsT=ident_bf[:],
                            rhs=gates_pre[:, t, 2 * p:2 * p + 2, :, :],
                            start=True, stop=False, skip_group_check=True)
                        for qi in range(2):
                            q = 2 * p + qi
                            for m in range(4):
                                j = 4 * m + q
                                for kt in range(KT):
                                    nc.tensor.matmul(
                                        out=Gp[:, qi, m, :],
                                        lhsT=whh_sb[:, kt,
                                                    j * 128:(j + 1) * 128],
                                        rhs=mm_rhs[:, kt, :],
                                        start=False, stop=(kt == KT - 1),
                                        skip_group_check=True)
                if probe == "mm":
                    # keep a minimal consumer so PSUM tiles recycle; no chains
                    for p in range(2):
                        nc.vector.tensor_copy(
                            ann_buf[:, slot, 2 * p:2 * p + 2, :],
                            g_tiles[p][:, :, 0, :])
                    h_prev = h0
                    if slot == slot_n - 1:
                        chunk = t // slot_n
                        nc.sync.dma_start(
                            out=ann_d[chunk, :, :],
                            in_=ann_buf[:].rearrange("p s q b -> p (s q b)"))
                    continue

                # elementwise, wave-major so chains pipeline across quad-pairs
                A = a_pool.tile([128, 4, 4, BS], F32)
                for p in range(2):
                    if probe == "ew":
                        Gp = s_pool.tile([128, 2, 4, BS], F32, tag="gdummy")
                        nc.vector.memset(Gp[:], 0.1)
                    else:
                        Gp = g_tiles[p]
                    # A[:,q,0]=i, [:,q,1]=f, [:,q,2]=sig(2g), [:,q,3]=o
                    nc.scalar.activation(
                        out=A[:, 2 * p:2 * p + 2, :, :], in_=Gp[:],
                        func=mybir.ActivationFunctionType.Sigmoid)
                gq = tmp_pool.tile([128, 4, BS], F32, tag="gq")
                for p in range(2):
                    sl = slice(2 * p, 2 * p + 2)
                    nc.vector.tensor_scalar(
                        out=gq[:, sl, :], in0=A[:, sl, 2, :],
                        scalar1=2.0, scalar2=1.0,
                        op0=mybir.AluOpType.mult, op1=mybir.AluOpType.subtract)
                ig = tmp_pool.tile([128, 4, BS], F32, tag="ig")
                for p in range(2):
                    sl = slice(2 * p, 2 * p + 2)
                    nc.vector.tensor_mul(ig[:, sl, :], A[:, sl, 0, :],
                                         gq[:, sl, :])
                fc = tmp_pool.tile([128, 4, BS], F32, tag="fc")
                for p in range(2):
                    sl = slice(2 * p, 2 * p + 2)
                    nc.vector.tensor_mul(fc[:, sl, :], A[:, sl, 1, :],
                                         C[:, sl, :])
                for p in range(2):
                    sl = slice(2 * p, 2 * p + 2)
                    nc.vector.tensor_add(C[:, sl, :], ig[:, sl, :],
                                         fc[:, sl, :])
                Tq = tmp_pool.tile([128, 4, BS], F32, tag="tq")
                for p in range(2):
                    sl = slice(2 * p, 2 * p + 2)
                    nc.scalar.activation(
                        out=Tq[:, sl, :], in_=C[:, sl, :],
                        func=mybir.ActivationFunctionType.Sigmoid, scale=2.0)
                t1 = tmp_pool.tile([128, 4, BS], F32, tag="t1")
                for p in range(2):
                    sl = slice(2 * p, 2 * p + 2)
                    nc.vector.tensor_scalar(
                        out=t1[:, sl, :], in0=Tq[:, sl, :],
                        scalar1=2.0, scalar2=1.0,
                        op0=mybir.AluOpType.mult, op1=mybir.AluOpType.subtract)
                for p in range(2):
                    sl = slice(2 * p, 2 * p + 2)
                    nc.vector.tensor_mul(ann_buf[:, slot, sl, :],
                                         A[:, sl, 3, :], t1[:, sl, :])

                h_prev = ann_buf[:, slot, :, :]

                if slot == slot_n - 1:
                    chunk = t // slot_n
                    nc.sync.dma_start(
                        out=ann_d[chunk, :, :],
                        in_=ann_buf[:].rearrange("p s q b -> p (s q b)"))

            nc.sync.dma_start(out=c_d[:, :],
                              in_=C[:].rearrange("p k b -> p (k b)"))
            loop_ctx.close()

    nc.finalize()
    return nc


_CACHE = {}


def _get_nc(seq=SEQ, probe=None):
    key = (str(LOOP_DT), seq, probe)
    if key not in _CACHE:
        _CACHE[key] = _build(LOOP_DT, seq, probe)
    return _CACHE[key]


LAST_EXEC_NS = None
LAST_RESULTS = None


def prep_in_maps(inputs, embedding, W_ih, W_hh, b_ih, b_hh, seq=SEQ):
    inputs = np.asarray(inputs)
    embedding = np.ascontiguousarray(np.asarray(embedding, dtype=np.float32))
    W_ih = np.asarray(W_ih, dtype=np.float32)
    W_hh = np.asarray(W_hh, dtype=np.float32)
    b_ih = np.asarray(b_ih, dtype=np.float32)
    b_hh = np.asarray(b_hh, dtype=np.float32)

    loop_np = np.float32 if LOOP_DT == F32 else ml_dtypes.bfloat16

    # W.T with the g-gate block (rows 2H:3H of W == cols of W.T) scaled by 2
    wih_t = W_ih.T.copy()
    wih_t[:, 2 * HID:3 * HID] *= 2.0
    whh_t = W_hh.T.copy()
    whh_t[:, 2 * HID:3 * HID] *= 2.0
    bias = (b_ih + b_hh).astype(np.float32)
    bias[2 * HID:3 * HID] *= 2.0
    biasj = np.ascontiguousarray(bias.reshape(MT, 128))

    wih_b = np.ascontiguousarray(wih_t.astype(ml_dtypes.bfloat16))
    whh_c = np.ascontiguousarray(whh_t.astype(loop_np))

    in_maps = []
    for r in range(NCORES):
        shard = inputs[r * BS:(r + 1) * BS, :seq].astype(np.int32)  # [BS, seq]
        flat = shard.T.reshape(-1)              # token n = t*BS + b
        idx = np.ascontiguousarray(flat.reshape(-1, 128).T)  # [128, ntiles]
        in_maps.append({
            "idx": idx,
            "emb": embedding,
            "wih": wih_b,
            "whh": whh_c,
            "biasj": biasj,
        })
    return in_maps


def unpack_outputs(results, seq=SEQ):
    slot_n = min(SLOT, seq)
    ann = np.empty((BATCH, seq, HID), dtype=np.float32)
    c_out = np.empty((BATCH, HID), dtype=np.float32)
    for r in range(NCORES):
        out = results[r]
        a = np.asarray(out["ann"]).astype(np.float32)
        a = a.reshape(-1, 128, slot_n, 4, BS)
        # a[c, p, s, q, b] -> ann[b, c*slot+s, q*128+p]
        ann[r * BS:(r + 1) * BS] = (
            a.transpose(4, 0, 2, 3, 1).reshape(BS, seq, HID))
        c = out["cout"].reshape(128, KT, BS)
        c_out[r * BS:(r + 1) * BS] = c.transpose(2, 1, 0).reshape(BS, HID)
    h_out = np.ascontiguousarray(ann[:, -1, :])
    return ann, h_out, c_out


def kernel(inputs, embedding, W_ih, W_hh, b_ih, b_hh, seq=SEQ, trace=False):
    global LAST_EXEC_NS, LAST_RESULTS
    in_maps = prep_in_maps(inputs, embedding, W_ih, W_hh, b_ih, b_hh, seq)
    nc = _get_nc(seq)
    res = bass_utils.run_bass_kernel_spmd(
        nc, in_maps, core_ids=list(range(NCORES)), trace=trace)
    LAST_EXEC_NS = res.exec_time_ns
    LAST_RESULTS = res
    return unpack_outputs(res.results, seq)


# revision 30
# speedup vs baseline: 1.2136x; 1.2136x over previous
"""LSTM encoder (embedding lookup + 512-step LSTMCell scan) on 8 trn2 cores.

Strategy: data-parallel over batch (8 rows/core, weights replicated).
  Prologue (per core):
    - indirect-DMA gather of embedding rows -> x tiles [128 tok, 512]
    - PE-transpose -> x.T (bf16), big matmul X @ W_ih.T + bias -> SBUF-resident
      bf16 gates_pre laid out [128p, t, q, m, b] (p=gate-row%128, q=h-quadrant,
      m=gate block i/f/g/o, b=batch)
  Recurrence (512 serial steps, per step):
    - 64 weights-stationary matmuls (lhsT = W_hh.T tile [128,128],
      rhs = h.T tile [128, 8]) accumulating gates.T into 4 PSUM tiles (one
      per h-quadrant, in separate PSUM banks)
    - elementwise in [128, 32] layout: S = psum + gates_pre; A = sigmoid(S)
      (the g-gate columns of W_hh.T/W_ih.T/bias are pre-scaled by 2 on the
      host so tanh(x) = 2*sigmoid(2x) - 1 needs no table switch);
      c = f*c + i*(2Ag-1); h = o*(2*sigmoid(2c)-1)
    - h.T lands directly in the layout needed as next step's moving operand.
Annotations accumulate in SBUF 16 steps at a time, DMA'd out per chunk.
Host reassembles the full (64, 512, 512) output + final h, c.
"""

import numpy as np
from contextlib import ExitStack
import ml_dtypes

import concourse.bass as bass
import concourse.mybir as mybir
import concourse.tile as tile
from concourse import bacc, bass_utils
from concourse.masks import make_identity
from concourse.tile_rust import add_dep_helper

VOCAB, HID, BATCH, SEQ = 32000, 512, 64, 512
NCORES = 8
BS = BATCH // NCORES          # 8 batch rows per core
G = 4 * HID                   # 2048 gate rows
KT = HID // 128               # 4 contraction tiles
MT = G // 128                 # 16 gate row-tiles
CHUNK_T = 64                  # timesteps per prologue chunk (=512 tokens)
NCHUNK = SEQ // CHUNK_T       # 8
SLOT = 16                     # steps per annotation DMA chunk
NANN = SEQ // SLOT            # 32 annotation chunks

F32 = mybir.dt.float32
BF16 = mybir.dt.bfloat16
I32 = mybir.dt.int32

# loop-matmul weight/h dtype: float32 (safe) or bfloat16 (fast LDWEIGHTS)
LOOP_DT = mybir.dt.bfloat16


def _build(loop_dt, seq=SEQ, probe=None):
    nchunk = seq // CHUNK_T if seq >= CHUNK_T else 1
    chunk_t = min(CHUNK_T, seq)
    nann = max(1, seq // SLOT)
    slot_n = min(SLOT, seq)

    nc = bacc.Bacc("TRN2", target_bir_lowering=False, debug=False,
                   enable_asserts=False, num_devices=NCORES)

    idx_d = nc.dram_tensor("idx", [128, (seq * BS) // 128], I32, kind="ExternalInput")
    emb_d = nc.dram_tensor("emb", [VOCAB, HID], F32, kind="ExternalInput")
    wih_d = nc.dram_tensor("wih", [HID, G], BF16, kind="ExternalInput")
    whh_d = nc.dram_tensor("whh", [HID, G], loop_dt, kind="ExternalInput")
    bias_d = nc.dram_tensor("biasj", [MT, 128], F32, kind="ExternalInput")
    ann_d = nc.dram_tensor("ann", [nann, 128, slot_n * 4 * BS], loop_dt,
                           kind="ExternalOutput")
    c_d = nc.dram_tensor("cout", [128, KT * BS], F32, kind="ExternalOutput")

    with tile.TileContext(nc) as tc:
        with tc.tile_pool(name="const", bufs=1) as const_pool:
            # ---- constants ----
            ident = const_pool.tile([128, 128], F32)
            make_identity(nc, ident)
            ident_bf = const_pool.tile([128, 128], BF16)
            nc.vector.tensor_copy(out=ident_bf[:], in_=ident[:])
            ntok_tiles = (seq * BS) // 128
            idx_sb = const_pool.tile([128, ntok_tiles], I32)
            nc.sync.dma_start(out=idx_sb[:], in_=idx_d[:, :])
            bias_sb = const_pool.tile([128, MT], F32)
            # bias_d is [MT, 128] in DRAM; land it as [128, MT] in SBUF
            nc.sync.dma_start(out=bias_sb[:], in_=bias_d.ap().rearrange("j p -> p j"))

            gates_pre = const_pool.tile([128, seq, 4, 4, BS], BF16)
            whh_sb = const_pool.tile([128, KT, G], loop_dt)

            C = const_pool.tile([128, KT, BS], F32)
            nc.vector.memset(C[:], 0.0)
            h0 = const_pool.tile([128, KT, BS], loop_dt)
            nc.vector.memset(h0[:], 0.0)

            # ---- prologue: gather + transpose + precompute ----
            with (
                tc.tile_pool(name="wih", bufs=1) as wih_pool,
                tc.tile_pool(name="gath", bufs=3) as gath_pool,
                tc.tile_pool(name="xt", bufs=2) as xt_pool,
                tc.tile_pool(name="tpsum", bufs=2, space="PSUM") as t_psum,
                tc.tile_pool(name="ppsum", bufs=2, space="PSUM") as p_psum,
            ):
                wih_sb = wih_pool.tile([128, KT, G], BF16)
                for kt in range(KT):
                    nc.sync.dma_start(out=wih_sb[:, kt, :],
                                      in_=wih_d[kt * 128:(kt + 1) * 128, :])

                for c in range(nchunk):
                    ntok = chunk_t * BS           # tokens in this chunk
                    gtiles = ntok // 128          # gather tiles (4)
                    xT = xt_pool.tile([128, KT, ntok], BF16)
                    for gl in range(gtiles):
                        xg = gath_pool.tile([128, HID], F32)
                        nc.gpsimd.indirect_dma_start(
                            out=xg[:],
                            out_offset=None,
                            in_=emb_d.ap(),
                            in_offset=bass.IndirectOffsetOnAxis(
                                ap=idx_sb[:, c * gtiles + gl:c * gtiles + gl + 1],
                                axis=0,
                            ),
                        )
                        for kt in range(KT):
                            ps = t_psum.tile([128, 128], F32)
                            nc.tensor.transpose(
                                out=ps[:], in_=xg[:, kt * 128:(kt + 1) * 128],
                                identity=ident[:])
                            nc.vector.tensor_copy(
                                out=xT[:, kt, gl * 128:(gl + 1) * 128], in_=ps[:])
                    for j in range(MT):
                        pj = p_psum.tile([128, chunk_t, BS], F32)
                        for kt in range(KT):
                            nc.tensor.matmul(
                                out=pj[:],
                                lhsT=wih_sb[:, kt, j * 128:(j + 1) * 128],
                                rhs=xT[:, kt, :],
                                start=(kt == 0), stop=(kt == KT - 1))
                        # evacuate with per-partition bias add, cast to bf16
                        nc.scalar.activation(
                            out=gates_pre[:, c * chunk_t:(c + 1) * chunk_t,
                                          j % 4, j // 4, :],
                            in_=pj[:],
                            func=mybir.ActivationFunctionType.Identity,
                            bias=bias_sb[:, j:j + 1],
                        )

            # load W_hh.T after prologue pools are done sizing the peak
            for kt in range(KT):
                nc.sync.dma_start(out=whh_sb[:, kt, :],
                                  in_=whh_d[kt * 128:(kt + 1) * 128, :])

            # ---- recurrence ----
            loop_ctx = ExitStack()
            ann_pool = loop_ctx.enter_context(tc.tile_pool(name="annp", bufs=2))
            s_pool = loop_ctx.enter_context(tc.tile_pool(name="sq", bufs=8))
            a_pool = loop_ctx.enter_context(tc.tile_pool(name="aq", bufs=8))
            tmp_pool = loop_ctx.enter_context(tc.tile_pool(name="tmp", bufs=12))
            g_pool = loop_ctx.enter_context(
                tc.tile_pool(name="gpsum", bufs=8, space="PSUM"))
            ann_buf = None
            h_prev = h0
            for t in range(seq):
                slot = t % slot_n
                if slot == 0:
                    ann_buf = ann_pool.tile([128, slot_n, 4, BS], loop_dt)

                g_tiles = []
                if probe != "ew":
                    mm_rhs = h0 if probe == "mm" else h_prev
                    for p in range(2):
                        Gp = g_pool.tile([128, 2, 4, BS], F32, space="PSUM")
                        g_tiles.append(Gp)
                        # seed PSUM with gates_pre via an identity matmul
                        # (start=True clears the whole bank's has_written and
                        # sets it for every element, so the W matmuls below
                        # accumulate with start=False). Needs no h, so the PE
                        # can run it while waiting on the recurrence.
                        nc.tensor.matmul(
                            out=Gp[:],
                            lhsT=ident_bf[:],
                            rhs=gates_pre[:, t, 2 * p:2 * p + 2, :, :],
                            start=True, stop=False, skip_group_check=True)
                        for qi in range(2):
                            q = 2 * p + qi
                            for m in range(4):
                                j = 4 * m + q
                                for kt in range(KT):
                                    nc.tensor.matmul(
                                        out=Gp[:, qi, m, :],
                                        lhsT=whh_sb[:, kt,
                                                    j * 128:(j + 1) * 128],
                                        rhs=mm_rhs[:, kt, :],
                                        start=False, stop=(kt == KT - 1),
                                        skip_group_check=True)
                if probe == "mm":
                    # keep a minimal consumer so PSUM tiles recycle; no chains
                    for p in range(2):
                        nc.vector.tensor_copy(
                            ann_buf[:, slot, 2 * p:2 * p + 2, :],
                            g_tiles[p][:, :, 0, :])
                    h_prev = h0
                    if slot == slot_n - 1:
                        chunk = t // slot_n
                        nc.sync.dma_start(
                            out=ann_d[chunk, :, :],
                            in_=ann_buf[:].rearrange("p s q b -> p (s q b)"))
                    continue

                # elementwise, wave-major so chains pipeline across quad-pairs
                A = a_pool.tile([128, 4, 4, BS], F32)
                for p in range(2):
                    if probe == "ew":
                        Gp = s_pool.tile([128, 2, 4, BS], F32, tag="gdummy")
                        nc.vector.memset(Gp[:], 0.1)
                    else:
                        Gp = g_tiles[p]
                    # A[:,q,0]=i, [:,q,1]=f, [:,q,2]=sig(2g), [:,q,3]=o
                    nc.scalar.activation(
                        out=A[:, 2 * p:2 * p + 2, :, :], in_=Gp[:],
                        func=mybir.ActivationFunctionType.Sigmoid)
                gq = tmp_pool.tile([128, 4, BS], F32, tag="gq")
                for p in range(2):
                    sl = slice(2 * p, 2 * p + 2)
                    nc.vector.tensor_scalar(
                        out=gq[:, sl, :], in0=A[:, sl, 2, :],
                        scalar1=2.0, scalar2=1.0,
                        op0=mybir.AluOpType.mult, op1=mybir.AluOpType.subtract)
                ig = tmp_pool.tile([128, 4, BS], F32, tag="ig")
                for p in range(2):
                    sl = slice(2 * p, 2 * p + 2)
                    nc.vector.tensor_mul(ig[:, sl, :], A[:, sl, 0, :],
                                         gq[:, sl, :])
                fc = tmp_pool.tile([128, 4, BS], F32, tag="fc")
                for p in range(2):
                    sl = slice(2 * p, 2 * p + 2)
                    nc.vector.tensor_mul(fc[:, sl, :], A[:, sl, 1, :],
                                         C[:, sl, :])
                for p in range(2):
                    sl = slice(2 * p, 2 * p + 2)
                    nc.vector.tensor_add(C[:, sl, :], ig[:, sl, :],
                                         fc[:, sl, :])
                t1 = tmp_pool.tile([128, 4, BS], F32, tag="t1")
                for p in range(2):
                    sl = slice(2 * p, 2 * p + 2)
                    nc.scalar.activation(
                        out=t1[:, sl, :], in_=C[:, sl, :],
                        func=mybir.ActivationFunctionType.Tanh)
                for p in range(2):
                    sl = slice(2 * p, 2 * p + 2)
                    nc.vector.tensor_mul(ann_buf[:, slot, sl, :],
                                         A[:, sl, 3, :], t1[:, sl, :])

                h_prev = ann_buf[:, slot, :, :]

                if slot == slot_n - 1:
                    chunk = t // slot_n
                    nc.sync.dma_start(
                        out=ann_d[chunk, :, :],
                        in_=ann_buf[:].rearrange("p s q b -> p (s q b)"))

            nc.sync.dma_start(out=c_d[:, :],
                              in_=C[:].rearrange("p k b -> p (k b)"))
            loop_ctx.close()

    nc.finalize()
    return nc


_CACHE = {}


def _get_nc(seq=SEQ, probe=None):
    key = (str(LOOP_DT), seq, probe)
    if key not in _CACHE:
        _CACHE[key] = _build(LOOP_DT, seq, probe)
    return _CACHE[key]


LAST_EXEC_NS = None
LAST_RESULTS = None


def prep_in_maps(inputs, embedding, W_ih, W_hh, b_ih, b_hh, seq=SEQ):
    inputs = np.asarray(inputs)
    embedding = np.ascontiguousarray(np.asarray(embedding, dtype=np.float32))
    W_ih = np.asarray(W_ih, dtype=np.float32)
    W_hh = np.asarray(W_hh, dtype=np.float32)
    b_ih = np.asarray(b_ih, dtype=np.float32)
    b_hh = np.asarray(b_hh, dtype=np.float32)

    loop_np = np.float32 if LOOP_DT == F32 else ml_dtypes.bfloat16

    # W.T with the g-gate block (rows 2H:3H of W == cols of W.T) scaled by 2
    wih_t = W_ih.T.copy()
    wih_t[:, 2 * HID:3 * HID] *= 2.0
    whh_t = W_hh.T.copy()
    whh_t[:, 2 * HID:3 * HID] *= 2.0
    bias = (b_ih + b_hh).astype(np.float32)
    bias[2 * HID:3 * HID] *= 2.0
    biasj = np.ascontiguousarray(bias.reshape(MT, 128))

    wih_b = np.ascontiguousarray(wih_t.astype(ml_dtypes.bfloat16))
    whh_c = np.ascontiguousarray(whh_t.astype(loop_np))

    in_maps = []
    for r in range(NCORES):
        shard = inputs[r * BS:(r + 1) * BS, :seq].astype(np.int32)  # [BS, seq]
        flat = shard.T.reshape(-1)              # token n = t*BS + b
        idx = np.ascontiguousarray(flat.reshape(-1, 128).T)  # [128, ntiles]
        in_maps.append({
            "idx": idx,
            "emb": embedding,
            "wih": wih_b,
            "whh": whh_c,
            "biasj": biasj,
        })
    return in_maps


def unpack_outputs(results, seq=SEQ):
    slot_n = min(SLOT, seq)
    ann = np.empty((BATCH, seq, HID), dtype=np.float32)
    c_out = np.empty((BATCH, HID), dtype=np.float32)
    for r in range(NCORES):
        out = results[r]
        a = np.asarray(out["ann"]).astype(np.float32)
        a = a.reshape(-1, 128, slot_n, 4, BS)
        # a[c, p, s, q, b] -> ann[b, c*slot+s, q*128+p]
        ann[r * BS:(r + 1) * BS] = (
            a.transpose(4, 0, 2, 3, 1).reshape(BS, seq, HID))
        c = out["cout"].reshape(128, KT, BS)
        c_out[r * BS:(r + 1) * BS] = c.transpose(2, 1, 0).reshape(BS, HID)
    h_out = np.ascontiguousarray(ann[:, -1, :])
    return ann, h_out, c_out


def kernel(inputs, embedding, W_ih, W_hh, b_ih, b_hh, seq=SEQ, trace=False):
    global LAST_EXEC_NS, LAST_RESULTS
    in_maps = prep_in_maps(inputs, embedding, W_ih, W_hh, b_ih, b_hh, seq)
    nc = _get_nc(seq)
    res = bass_utils.run_bass_kernel_spmd(
        nc, in_maps, core_ids=list(range(NCORES)), trace=trace)
    LAST_EXEC_NS = res.exec_time_ns
    LAST_RESULTS = res
    return unpack_outputs(res.results, seq)


# revision 31
# speedup vs baseline: 1.2290x; 1.0126x over previous
"""LSTM encoder (embedding lookup + 512-step LSTMCell scan) on 8 trn2 cores.

Strategy: data-parallel over batch (8 rows/core, weights replicated).
  Prologue (per core):
    - indirect-DMA gather of embedding rows -> x tiles [128 tok, 512]
    - PE-transpose -> x.T (bf16), big matmul X @ W_ih.T + bias -> SBUF-resident
      bf16 gates_pre laid out [128p, t, q, m, b] (p=gate-row%128, q=h-quadrant,
      m=gate block i/f/g/o, b=batch)
  Recurrence (512 serial steps, per step):
    - 64 weights-stationary matmuls (lhsT = W_hh.T tile [128,128],
      rhs = h.T tile [128, 8]) accumulating gates.T into 4 PSUM tiles (one
      per h-quadrant, in separate PSUM banks)
    - elementwise in [128, 32] layout: S = psum + gates_pre; A = sigmoid(S)
      (the g-gate columns of W_hh.T/W_ih.T/bias are pre-scaled by 2 on the
      host so tanh(x) = 2*sigmoid(2x) - 1 needs no table switch);
      c = f*c + i*(2Ag-1); h = o*(2*sigmoid(2c)-1)
    - h.T lands directly in the layout needed as next step's moving operand.
Annotations accumulate in SBUF 16 steps at a time, DMA'd out per chunk.
Host reassembles the full (64, 512, 512) output + final h, c.
"""

import numpy as np
from contextlib import ExitStack
import ml_dtypes

import concourse.bass as bass
import concourse.mybir as mybir
import concourse.tile as tile
from concourse import bacc, bass_utils
from concourse.masks import make_identity
from concourse.tile_rust import add_dep_helper

VOCAB, HID, BATCH, SEQ = 32000, 512, 64, 512
NCORES = 8
BS = BATCH // NCORES          # 8 batch rows per core
G = 4 * HID                   # 2048 gate rows
KT = HID // 128               # 4 contraction tiles
MT = G // 128                 # 16 gate row-tiles
CHUNK_T = 64                  # timesteps per prologue chunk (=512 tokens)
NCHUNK = SEQ // CHUNK_T       # 8
SLOT = 16                     # steps per annotation DMA chunk
NANN = SEQ // SLOT            # 32 annotation chunks

F32 = mybir.dt.float32
BF16 = mybir.dt.bfloat16
I32 = mybir.dt.int32

# loop-matmul weight/h dtype: float32 (safe) or bfloat16 (fast LDWEIGHTS)
LOOP_DT = mybir.dt.bfloat16


def _build(loop_dt, seq=SEQ, probe=None):
    nchunk = seq // CHUNK_T if seq >= CHUNK_T else 1
    chunk_t = min(CHUNK_T, seq)
    nann = max(1, seq // SLOT)
    slot_n = min(SLOT, seq)

    nc = bacc.Bacc("TRN2", target_bir_lowering=False, debug=False,
                   enable_asserts=False, num_devices=NCORES)

    idx_d = nc.dram_tensor("idx", [128, (seq * BS) // 128], I32, kind="ExternalInput")
    emb_d = nc.dram_tensor("emb", [VOCAB, HID], F32, kind="ExternalInput")
    wih_d = nc.dram_tensor("wih", [HID, G], BF16, kind="ExternalInput")
    whh_d = nc.dram_tensor("whh", [HID, G], loop_dt, kind="ExternalInput")
    bias_d = nc.dram_tensor("biasj", [MT, 128], F32, kind="ExternalInput")
    ann_d = nc.dram_tensor("ann", [nann, 128, slot_n * 4 * BS], loop_dt,
                           kind="ExternalOutput")
    c_d = nc.dram_tensor("cout", [128, KT * BS], F32, kind="ExternalOutput")

    with tile.TileContext(nc) as tc:
        with tc.tile_pool(name="const", bufs=1) as const_pool:
            # ---- constants ----
            ident = const_pool.tile([128, 128], F32)
            make_identity(nc, ident)
            ident_bf = const_pool.tile([128, 128], BF16)
            nc.vector.tensor_copy(out=ident_bf[:], in_=ident[:])
            ntok_tiles = (seq * BS) // 128
            idx_sb = const_pool.tile([128, ntok_tiles], I32)
            nc.sync.dma_start(out=idx_sb[:], in_=idx_d[:, :])
            bias_sb = const_pool.tile([128, MT], F32)
            # bias_d is [MT, 128] in DRAM; land it as [128, MT] in SBUF
            nc.sync.dma_start(out=bias_sb[:], in_=bias_d.ap().rearrange("j p -> p j"))

            gates_pre = const_pool.tile([128, seq, 4, 4, BS], BF16)
            whh_sb = const_pool.tile([128, KT, G], loop_dt)

            C = const_pool.tile([128, KT, BS], F32)
            nc.vector.memset(C[:], 0.0)
            h0 = const_pool.tile([128, KT, BS], loop_dt)
            nc.vector.memset(h0[:], 0.0)

            # ---- prologue: gather + transpose + precompute ----
            with (
                tc.tile_pool(name="wih", bufs=1) as wih_pool,
                tc.tile_pool(name="gath", bufs=3) as gath_pool,
                tc.tile_pool(name="xt", bufs=2) as xt_pool,
                tc.tile_pool(name="tpsum", bufs=2, space="PSUM") as t_psum,
                tc.tile_pool(name="ppsum", bufs=2, space="PSUM") as p_psum,
            ):
                wih_sb = wih_pool.tile([128, KT, G], BF16)
                for kt in range(KT):
                    nc.sync.dma_start(out=wih_sb[:, kt, :],
                                      in_=wih_d[kt * 128:(kt + 1) * 128, :])

                for c in range(nchunk):
                    ntok = chunk_t * BS           # tokens in this chunk
                    gtiles = ntok // 128          # gather tiles (4)
                    xT = xt_pool.tile([128, KT, ntok], BF16)
                    for gl in range(gtiles):
                        xg = gath_pool.tile([128, HID], F32)
                        nc.gpsimd.indirect_dma_start(
                            out=xg[:],
                            out_offset=None,
                            in_=emb_d.ap(),
                            in_offset=bass.IndirectOffsetOnAxis(
                                ap=idx_sb[:, c * gtiles + gl:c * gtiles + gl + 1],
                                axis=0,
                            ),
                        )
                        for kt in range(KT):
                            ps = t_psum.tile([128, 128], F32)
                            nc.tensor.transpose(
                                out=ps[:], in_=xg[:, kt * 128:(kt + 1) * 128],
                                identity=ident[:])
                            nc.vector.tensor_copy(
                                out=xT[:, kt, gl * 128:(gl + 1) * 128], in_=ps[:])
                    for j in range(MT):
                        pj = p_psum.tile([128, chunk_t, BS], F32)
                        for kt in range(KT):
                            nc.tensor.matmul(
                                out=pj[:],
                                lhsT=wih_sb[:, kt, j * 128:(j + 1) * 128],
                                rhs=xT[:, kt, :],
                                start=(kt == 0), stop=(kt == KT - 1))
                        # evacuate with per-partition bias add, cast to bf16
                        nc.scalar.activation(
                            out=gates_pre[:, c * chunk_t:(c + 1) * chunk_t,
                                          j % 4, j // 4, :],
                            in_=pj[:],
                            func=mybir.ActivationFunctionType.Identity,
                            bias=bias_sb[:, j:j + 1],
                        )

            # load W_hh.T after prologue pools are done sizing the peak
            for kt in range(KT):
                nc.sync.dma_start(out=whh_sb[:, kt, :],
                                  in_=whh_d[kt * 128:(kt + 1) * 128, :])

            # ---- recurrence ----
            loop_ctx = ExitStack()
            ann_pool = loop_ctx.enter_context(tc.tile_pool(name="annp", bufs=2))
            s_pool = loop_ctx.enter_context(tc.tile_pool(name="sq", bufs=8))
            a_pool = loop_ctx.enter_context(tc.tile_pool(name="aq", bufs=8))
            tmp_pool = loop_ctx.enter_context(tc.tile_pool(name="tmp", bufs=12))
            g_pool = loop_ctx.enter_context(
                tc.tile_pool(name="gpsum", bufs=8, space="PSUM"))
            ann_buf = None
            h_prev = h0
            for t in range(seq):
                slot = t % slot_n
                if slot == 0:
                    ann_buf = ann_pool.tile([128, slot_n, 4, BS], loop_dt)

                g_tiles = []
                if probe != "ew":
                    mm_rhs = h0 if probe == "mm" else h_prev
                    for p in range(2):
                        Gp = g_pool.tile([128, 2, 4, BS], F32, space="PSUM")
                        g_tiles.append(Gp)
                        # seed PSUM with gates_pre via an identity matmul
                        # (start=True clears the whole bank's has_written and
                        # sets it for every element, so the W matmuls below
                        # accumulate with start=False). Needs no h, so the PE
                        # can run it while waiting on the recurrence.
                        nc.tensor.matmul(
                            out=Gp[:],
                            lhsT=ident_bf[:],
                            rhs=gates_pre[:, t, 2 * p:2 * p + 2, :, :],
                            start=True, stop=False, skip_group_check=True)
                        for qi in range(2):
                            q = 2 * p + qi
                            for m in range(4):
                                j = 4 * m + q
                                for kt in range(KT):
                                    nc.tensor.matmul(
                                        out=Gp[:, qi, m, :],
                                        lhsT=whh_sb[:, kt,
                                                    j * 128:(j + 1) * 128],
                                        rhs=mm_rhs[:, kt, :],
                                        start=False, stop=(kt == KT - 1),
                                        skip_group_check=True)
                if probe == "mm":
                    # keep a minimal consumer so PSUM tiles recycle; no chains
                    for p in range(2):
                        nc.vector.tensor_copy(
                            ann_buf[:, slot, 2 * p:2 * p + 2, :],
                            g_tiles[p][:, :, 0, :])
                    h_prev = h0
                    if slot == slot_n - 1:
                        chunk = t // slot_n
                        nc.sync.dma_start(
                            out=ann_d[chunk, :, :],
                            in_=ann_buf[:].rearrange("p s q b -> p (s q b)"))
                    continue

                # elementwise, wave-major so chains pipeline across quad-pairs
                A = a_pool.tile([128, 4, 4, BS], F32)
                for p in range(2):
                    if probe == "ew":
                        Gp = s_pool.tile([128, 2, 4, BS], F32, tag="gdummy")
                        nc.vector.memset(Gp[:], 0.1)
                    else:
                        Gp = g_tiles[p]
                    # A[:,q,0]=i, [:,q,1]=f, [:,q,2]=sig(2g), [:,q,3]=o
                    nc.scalar.activation(
                        out=A[:, 2 * p:2 * p + 2, :, :], in_=Gp[:],
                        func=mybir.ActivationFunctionType.Sigmoid)
                # chain-major per pair: pair0's h completes as early as
                # possible to unblock the next step's matmuls
                gq = tmp_pool.tile([128, 4, BS], F32, tag="gq")
                ig = tmp_pool.tile([128, 4, BS], F32, tag="ig")
                fc = tmp_pool.tile([128, 4, BS], F32, tag="fc")
                t1 = tmp_pool.tile([128, 4, BS], F32, tag="t1")
                for p in range(2):
                    sl = slice(2 * p, 2 * p + 2)
                    nc.vector.tensor_scalar(
                        out=gq[:, sl, :], in0=A[:, sl, 2, :],
                        scalar1=2.0, scalar2=1.0,
                        op0=mybir.AluOpType.mult, op1=mybir.AluOpType.subtract)
                    nc.vector.tensor_mul(ig[:, sl, :], A[:, sl, 0, :],
                                         gq[:, sl, :])
                    nc.vector.tensor_mul(fc[:, sl, :], A[:, sl, 1, :],
                                         C[:, sl, :])
                    nc.vector.tensor_add(C[:, sl, :], ig[:, sl, :],
                                         fc[:, sl, :])
                    nc.scalar.activation(
                        out=t1[:, sl, :], in_=C[:, sl, :],
                        func=mybir.ActivationFunctionType.Tanh)
                    nc.vector.tensor_mul(ann_buf[:, slot, sl, :],
                                         A[:, sl, 3, :], t1[:, sl, :])

                h_prev = ann_buf[:, slot, :, :]

                if slot == slot_n - 1:
                    chunk = t // slot_n
                    nc.sync.dma_start(
                        out=ann_d[chunk, :, :],
                        in_=ann_buf[:].rearrange("p s q b -> p (s q b)"))

            nc.sync.dma_start(out=c_d[:, :],
                              in_=C[:].rearrange("p k b -> p (k b)"))
            loop_ctx.close()

    nc.finalize()
    return nc


_CACHE = {}


def _get_nc(seq=SEQ, probe=None):
    key = (str(LOOP_DT), seq, probe)
    if key not in _CACHE:
        _CACHE[key] = _build(LOOP_DT, seq, probe)
    return _CACHE[key]


LAST_EXEC_NS = None
LAST_RESULTS = None


def prep_in_maps(inputs, embedding, W_ih, W_hh, b_ih, b_hh, seq=SEQ):
    inputs = np.asarray(inputs)
    embedding = np.ascontiguousarray(np.asarray(embedding, dtype=np.float32))
    W_ih = np.asarray(W_ih, dtype=np.float32)
    W_hh = np.asarray(W_hh, dtype=np.float32)
    b_ih = np.asarray(b_ih, dtype=np.float32)
    b_hh = np.asarray(b_hh, dtype=np.float32)

    loop_np = np.float32 if LOOP_DT == F32 else ml_dtypes.bfloat16

    # W.T with the g-gate block (rows 2H:3H of W == cols of W.T) scaled by 2
    wih_t = W_ih.T.copy()
    wih_t[:, 2 * HID:3 * HID] *= 2.0
    whh_t = W_hh.T.copy()
    whh_t[:, 2 * HID:3 * HID] *= 2.0
    bias = (b_ih + b_hh).astype(np.float32)
    bias[2 * HID:3 * HID] *= 2.0
    biasj = np.ascontiguousarray(bias.reshape(MT, 128))

    wih_b = np.ascontiguousarray(wih_t.astype(ml_dtypes.bfloat16))
    whh_c = np.ascontiguousarray(whh_t.astype(loop_np))

    in_maps = []
    for r in range(NCORES):
        shard = inputs[r * BS:(r + 1) * BS, :seq].astype(np.int32)  # [BS, seq]
        flat = shard.T.reshape(-1)              # token n = t*BS + b
        idx = np.ascontiguousarray(flat.reshape(-1, 128).T)  # [128, ntiles]
        in_maps.append({
            "idx": idx,
            "emb": embedding,
            "wih": wih_b,
            "whh": whh_c,
            "biasj": biasj,
        })
    return in_maps


def unpack_outputs(results, seq=SEQ):
    slot_n = min(SLOT, seq)
    ann = np.empty((BATCH, seq, HID), dtype=np.float32)
    c_out = np.empty((BATCH, HID), dtype=np.float32)
    for r in range(NCORES):
        out = results[r]
        a = np.asarray(out["ann"]).astype(np.float32)
        a = a.reshape(-1, 128, slot_n, 4, BS)
        # a[c, p, s, q, b] -> ann[b, c*slot+s, q*128+p]
        ann[r * BS:(r + 1) * BS] = (
            a.transpose(4, 0, 2, 3, 1).reshape(BS, seq, HID))
        c = out["cout"].reshape(128, KT, BS)
        c_out[r * BS:(r + 1) * BS] = c.transpose(2, 1, 0).reshape(BS, HID)
    h_out = np.ascontiguousarray(ann[:, -1, :])
    return ann, h_out, c_out


def kernel(inputs, embedding, W_ih, W_hh, b_ih, b_hh, seq=SEQ, trace=False):
    global LAST_EXEC_NS, LAST_RESULTS
    in_maps = prep_in_maps(inputs, embedding, W_ih, W_hh, b_ih, b_hh, seq)
    nc = _get_nc(seq)
    res = bass_utils.run_bass_kernel_spmd(
        nc, in_maps, core_ids=list(range(NCORES)), trace=trace)
    LAST_EXEC_NS = res.exec_time_ns
    LAST_RESULTS = res
    return unpack_outputs(res.results, seq)


# revision 35
# speedup vs baseline: 41.5583x; 33.8159x over previous
"""LSTM encoder (embedding lookup + 512-step LSTMCell scan) on 8 trn2 cores.

Strategy: data-parallel over batch (8 rows/core, weights replicated).
  Prologue (per core):
    - indirect-DMA gather of embedding rows -> x tiles [128 tok, 512]
    - PE-transpose -> x.T (bf16), big matmul X @ W_ih.T + bias -> SBUF-resident
      bf16 gates_pre laid out [128p, t, q, m, b] (p=gate-row%128, q=h-quadrant,
      m=gate block i/f/g/o, b=batch)
  Recurrence (512 serial steps, per step):
    - 64 weights-stationary matmuls (lhsT = W_hh.T tile [128,128],
      rhs = h.T tile [128, 8]) accumulating gates.T into 4 PSUM tiles (one
      per h-quadrant, in separate PSUM banks)
    - elementwise in [128, 32] layout: S = psum + gates_pre; A = sigmoid(S)
      (the g-gate columns of W_hh.T/W_ih.T/bias are pre-scaled by 2 on the
      host so tanh(x) = 2*sigmoid(2x) - 1 needs no table switch);
      c = f*c + i*(2Ag-1); h = o*(2*sigmoid(2c)-1)
    - h.T lands directly in the layout needed as next step's moving operand.
Annotations accumulate in SBUF 16 steps at a time, DMA'd out per chunk.
Host reassembles the full (64, 512, 512) output + final h, c.
"""

import numpy as np
from contextlib import ExitStack
import ml_dtypes

import concourse.bass as bass
import concourse.mybir as mybir
import concourse.tile as tile
from concourse import bacc, bass_utils
from concourse.masks import make_identity

VOCAB, HID, BATCH, SEQ = 32000, 512, 64, 512
NCORES = 8
BS = BATCH // NCORES          # 8 batch rows per core
G = 4 * HID                   # 2048 gate rows
KT = HID // 128               # 4 contraction tiles
MT = G // 128                 # 16 gate row-tiles
CHUNK_T = 64                  # timesteps per prologue chunk (=512 tokens)
NCHUNK = SEQ // CHUNK_T       # 8
SLOT = 16                     # steps per annotation DMA chunk
NANN = SEQ // SLOT            # 32 annotation chunks

F32 = mybir.dt.float32
BF16 = mybir.dt.bfloat16
I32 = mybir.dt.int32

# loop-matmul weight/h dtype: float32 (safe) or bfloat16 (fast LDWEIGHTS)
LOOP_DT = mybir.dt.bfloat16
# emission order of the per-step elementwise chains: True = chain-major per
# pair (early h for pair 0), False = wave-major across pairs
CHAIN_MAJOR = True


def _build(loop_dt, seq=SEQ, probe=None):
    nchunk = seq // CHUNK_T if seq >= CHUNK_T else 1
    chunk_t = min(CHUNK_T, seq)
    nann = max(1, seq // SLOT)
    slot_n = min(SLOT, seq)

    nc = bacc.Bacc("TRN2", target_bir_lowering=False, debug=False,
                   enable_asserts=False, num_devices=NCORES)

    idx_d = nc.dram_tensor("idx", [128, (seq * BS) // 128], I32, kind="ExternalInput")
    emb_d = nc.dram_tensor("emb", [VOCAB, HID], F32, kind="ExternalInput")
    wih_d = nc.dram_tensor("wih", [HID, G], BF16, kind="ExternalInput")
    whh_d = nc.dram_tensor("whh", [HID, G], loop_dt, kind="ExternalInput")
    bias_d = nc.dram_tensor("biasj", [MT, 128], F32, kind="ExternalInput")
    ann_d = nc.dram_tensor("ann", [nann, 128, slot_n * 4 * BS], loop_dt,
                           kind="ExternalOutput")
    c_d = nc.dram_tensor("cout", [128, KT * BS], F32, kind="ExternalOutput")

    with tile.TileContext(nc) as tc:
        with tc.tile_pool(name="const", bufs=1) as const_pool:
            # ---- constants ----
            ident = const_pool.tile([128, 128], F32)
            make_identity(nc, ident)
            ident_bf = const_pool.tile([128, 128], BF16)
            nc.vector.tensor_copy(out=ident_bf[:], in_=ident[:])
            ntok_tiles = (seq * BS) // 128
            idx_sb = const_pool.tile([128, ntok_tiles], I32)
            nc.sync.dma_start(out=idx_sb[:], in_=idx_d[:, :])
            bias_sb = const_pool.tile([128, MT], F32)
            # bias_d is [MT, 128] in DRAM; land it as [128, MT] in SBUF
            nc.sync.dma_start(out=bias_sb[:], in_=bias_d.ap().rearrange("j p -> p j"))

            gates_pre = const_pool.tile([128, seq, 4, 4, BS], BF16)
            whh_sb = const_pool.tile([128, KT, G], loop_dt)

            C = const_pool.tile([128, KT, BS], F32)
            nc.vector.memset(C[:], 0.0)
            h0 = const_pool.tile([128, KT, BS], loop_dt)
            nc.vector.memset(h0[:], 0.0)

            # ---- prologue: gather + transpose + precompute ----
            with (
                tc.tile_pool(name="wih", bufs=1) as wih_pool,
                tc.tile_pool(name="gath", bufs=3) as gath_pool,
                tc.tile_pool(name="xt", bufs=2) as xt_pool,
                tc.tile_pool(name="tpsum", bufs=2, space="PSUM") as t_psum,
                tc.tile_pool(name="ppsum", bufs=2, space="PSUM") as p_psum,
            ):
                wih_sb = wih_pool.tile([128, KT, G], BF16)
                for kt in range(KT):
                    nc.sync.dma_start(out=wih_sb[:, kt, :],
                                      in_=wih_d[kt * 128:(kt + 1) * 128, :])

                for c in range(nchunk):
                    ntok = chunk_t * BS           # tokens in this chunk
                    gtiles = ntok // 128          # gather tiles (4)
                    xT = xt_pool.tile([128, KT, ntok], BF16)
                    for gl in range(gtiles):
                        xg = gath_pool.tile([128, HID], F32)
                        nc.gpsimd.indirect_dma_start(
                            out=xg[:],
                            out_offset=None,
                            in_=emb_d.ap(),
                            in_offset=bass.IndirectOffsetOnAxis(
                                ap=idx_sb[:, c * gtiles + gl:c * gtiles + gl + 1],
                                axis=0,
                            ),
                        )
                        for kt in range(KT):
                            ps = t_psum.tile([128, 128], F32)
                            nc.tensor.transpose(
                                out=ps[:], in_=xg[:, kt * 128:(kt + 1) * 128],
                                identity=ident[:])
                            nc.vector.tensor_copy(
                                out=xT[:, kt, gl * 128:(gl + 1) * 128], in_=ps[:])
                    for j in range(MT):
                        pj = p_psum.tile([128, chunk_t, BS], F32)
                        for kt in range(KT):
                            nc.tensor.matmul(
                                out=pj[:],
                                lhsT=wih_sb[:, kt, j * 128:(j + 1) * 128],
                                rhs=xT[:, kt, :],
                                start=(kt == 0), stop=(kt == KT - 1))
                        # evacuate with per-partition bias add, cast to bf16
                        nc.scalar.activation(
                            out=gates_pre[:, c * chunk_t:(c + 1) * chunk_t,
                                          j % 4, j // 4, :],
                            in_=pj[:],
                            func=mybir.ActivationFunctionType.Identity,
                            bias=bias_sb[:, j:j + 1],
                        )

            # load W_hh.T after prologue pools are done sizing the peak
            for kt in range(KT):
                nc.sync.dma_start(out=whh_sb[:, kt, :],
                                  in_=whh_d[kt * 128:(kt + 1) * 128, :])

            # ---- recurrence ----
            loop_ctx = ExitStack()
            ann_pool = loop_ctx.enter_context(tc.tile_pool(name="annp", bufs=2))
            s_pool = loop_ctx.enter_context(tc.tile_pool(name="sq", bufs=8))
            a_pool = loop_ctx.enter_context(tc.tile_pool(name="aq", bufs=8))
            tmp_pool = loop_ctx.enter_context(tc.tile_pool(name="tmp", bufs=12))
            g_pool = loop_ctx.enter_context(
                tc.tile_pool(name="gpsum", bufs=8, space="PSUM"))
            ann_buf = None
            h_prev = h0
            for t in range(seq):
                slot = t % slot_n
                if slot == 0:
                    ann_buf = ann_pool.tile([128, slot_n, 4, BS], loop_dt)

                g_tiles = []
                if probe != "ew":
                    mm_rhs = h0 if probe == "mm" else h_prev
                    for p in range(2):
                        Gp = g_pool.tile([128, 2, 4, BS], F32, space="PSUM")
                        g_tiles.append(Gp)
                        # seed PSUM with gates_pre via an identity matmul
                        # (start=True clears the whole bank's has_written and
                        # sets it for every element, so the W matmuls below
                        # accumulate with start=False). Needs no h, so the PE
                        # can run it while waiting on the recurrence.
                        nc.tensor.matmul(
                            out=Gp[:],
                            lhsT=ident_bf[:],
                            rhs=gates_pre[:, t, 2 * p:2 * p + 2, :, :],
                            start=True, stop=False, skip_group_check=True)
                        for qi in range(2):
                            q = 2 * p + qi
                            for m in range(4):
                                j = 4 * m + q
                                for kt in range(KT):
                                    nc.tensor.matmul(
                                        out=Gp[:, qi, m, :],
                                        lhsT=whh_sb[:, kt,
                                                    j * 128:(j + 1) * 128],
                                        rhs=mm_rhs[:, kt, :],
                                        start=False, stop=(kt == KT - 1),
                                        skip_group_check=True)
                if probe == "mm":
                    # keep a minimal consumer so PSUM tiles recycle; no chains
                    for p in range(2):
                        nc.vector.tensor_copy(
                            ann_buf[:, slot, 2 * p:2 * p + 2, :],
                            g_tiles[p][:, :, 0, :])
                    h_prev = h0
                    if slot == slot_n - 1:
                        chunk = t // slot_n
                        nc.sync.dma_start(
                            out=ann_d[chunk, :, :],
                            in_=ann_buf[:].rearrange("p s q b -> p (s q b)"))
                    continue

                # elementwise, wave-major so chains pipeline across quad-pairs
                A = a_pool.tile([128, 4, 4, BS], F32)
                for p in range(2):
                    if probe == "ew":
                        Gp = s_pool.tile([128, 2, 4, BS], F32, tag="gdummy")
                        nc.vector.memset(Gp[:], 0.1)
                    else:
                        Gp = g_tiles[p]
                    # A[:,q,0]=i, [:,q,1]=f, [:,q,2]=sig(2g), [:,q,3]=o
                    nc.scalar.activation(
                        out=A[:, 2 * p:2 * p + 2, :, :], in_=Gp[:],
                        func=mybir.ActivationFunctionType.Sigmoid)
                # pair0's h should complete as early as possible to unblock
                # the next step's matmuls
                gq = tmp_pool.tile([128, 4, BS], F32, tag="gq")
                ig = tmp_pool.tile([128, 4, BS], F32, tag="ig")
                fc = tmp_pool.tile([128, 4, BS], F32, tag="fc")
                t1 = tmp_pool.tile([128, 4, BS], F32, tag="t1")

                def chain(p, ops):
                    sl = slice(2 * p, 2 * p + 2)
                    steps = {
                        "gq": lambda: nc.vector.tensor_scalar(
                            out=gq[:, sl, :], in0=A[:, sl, 2, :],
                            scalar1=2.0, scalar2=1.0,
                            op0=mybir.AluOpType.mult,
                            op1=mybir.AluOpType.subtract),
                        "ig": lambda: nc.vector.tensor_mul(
                            ig[:, sl, :], A[:, sl, 0, :], gq[:, sl, :]),
                        "fc": lambda: nc.vector.tensor_mul(
                            fc[:, sl, :], A[:, sl, 1, :], C[:, sl, :]),
                        "c": lambda: nc.vector.tensor_add(
                            C[:, sl, :], ig[:, sl, :], fc[:, sl, :]),
                        "t1": lambda: nc.scalar.activation(
                            out=t1[:, sl, :], in_=C[:, sl, :],
                            func=mybir.ActivationFunctionType.Tanh),
                        "h": lambda: nc.vector.tensor_mul(
                            ann_buf[:, slot, sl, :], A[:, sl, 3, :],
                            t1[:, sl, :]),
                    }
                    for op in ops:
                        steps[op]()

                if CHAIN_MAJOR:
                    for p in range(2):
                        chain(p, ["gq", "ig", "fc", "c", "t1", "h"])
                else:
                    for op in ["gq", "ig", "fc", "c", "t1", "h"]:
                        for p in range(2):
                            chain(p, [op])

                h_prev = ann_buf[:, slot, :, :]

                if slot == slot_n - 1:
                    chunk = t // slot_n
                    nc.sync.dma_start(
                        out=ann_d[chunk, :, :],
                        in_=ann_buf[:].rearrange("p s q b -> p (s q b)"))

            nc.sync.dma_start(out=c_d[:, :],
                              in_=C[:].rearrange("p k b -> p (k b)"))
            loop_ctx.close()

    nc.finalize()
    return nc


_CACHE = {}


def _get_nc(seq=SEQ, probe=None):
    key = (str(LOOP_DT), seq, probe, CHAIN_MAJOR)
    if key not in _CACHE:
        _CACHE[key] = _build(LOOP_DT, seq, probe)
    return _CACHE[key]


LAST_EXEC_NS = None
LAST_RESULTS = None


def prep_in_maps(inputs, embedding, W_ih, W_hh, b_ih, b_hh, seq=SEQ):
    inputs = np.asarray(inputs)
    embedding = np.ascontiguousarray(np.asarray(embedding, dtype=np.float32))
    W_ih = np.asarray(W_ih, dtype=np.float32)
    W_hh = np.asarray(W_hh, dtype=np.float32)
    b_ih = np.asarray(b_ih, dtype=np.float32)
    b_hh = np.asarray(b_hh, dtype=np.float32)

    loop_np = np.float32 if LOOP_DT == F32 else ml_dtypes.bfloat16

    # W.T with the g-gate block (rows 2H:3H of W == cols of W.T) scaled by 2
    wih_t = W_ih.T.copy()
    wih_t[:, 2 * HID:3 * HID] *= 2.0
    whh_t = W_hh.T.copy()
    whh_t[:, 2 * HID:3 * HID] *= 2.0
    bias = (b_ih + b_hh).astype(np.float32)
    bias[2 * HID:3 * HID] *= 2.0
    biasj = np.ascontiguousarray(bias.reshape(MT, 128))

    wih_b = np.ascontiguousarray(wih_t.astype(ml_dtypes.bfloat16))
    whh_c = np.ascontiguousarray(whh_t.astype(loop_np))

    in_maps = []
    for r in range(NCORES):
        shard = inputs[r * BS:(r + 1) * BS, :seq].astype(np.int32)  # [BS, seq]
        flat = shard.T.reshape(-1)              # token n = t*BS + b
        idx = np.ascontiguousarray(flat.reshape(-1, 128).T)  # [128, ntiles]
        in_maps.append({
            "idx": idx,
            "emb": embedding,
            "wih": wih_b,
            "whh": whh_c,
            "biasj": biasj,
        })
    return in_maps


def unpack_outputs(results, seq=SEQ):
    slot_n = min(SLOT, seq)
    ann = np.empty((BATCH, seq, HID), dtype=np.float32)
    c_out = np.empty((BATCH, HID), dtype=np.float32)
    for r in range(NCORES):
        out = results[r]
        a = np.asarray(out["ann"]).astype(np.float32)
        a = a.reshape(-1, 128, slot_n, 4, BS)
        # a[c, p, s, q, b] -> ann[b, c*slot+s, q*128+p]
        ann[r * BS:(r + 1) * BS] = (
            a.transpose(4, 0, 2, 3, 1).reshape(BS, seq, HID))
        c = out["cout"].reshape(128, KT, BS)
        c_out[r * BS:(r + 1) * BS] = c.transpose(2, 1, 0).reshape(BS, HID)
    h_out = np.ascontiguousarray(ann[:, -1, :])
    return ann, h_out, c_out


def kernel(inputs, embedding, W_ih, W_hh, b_ih, b_hh, seq=SEQ, trace=False):
    global LAST_EXEC_NS, LAST_RESULTS
    in_maps = prep_in_maps(inputs, embedding, W_ih, W_hh, b_ih, b_hh, seq)
    nc = _get_nc(seq)
    res = bass_utils.run_bass_kernel_spmd(
        nc, in_maps, core_ids=list(range(NCORES)), trace=trace)
    LAST_EXEC_NS = res.exec_time_ns
    LAST_RESULTS = res
    return unpack_outputs(res.results, seq)
